# revision 5
# baseline (speedup 1.0000x reference)
"""Trainium2 Bass kernel for nn_KnowledgeBaseLookup (bucketed dma_gather design).

Computation (see reference):
    lookup = knowledge_base[indexes]            # (B,T,K,D) gather
    y      = einsum('btk,btkd->btd', weights, lookup)
    out    = y @ w_out.T + b_out                # (B,T,E)

Sharding: data-parallel over the B*T token dim across 8 cores; the
knowledge_base table is replicated per core.

Per-core design (1024 tokens, 16384 gathered rows):
  The old per-slab indirect-DMA gather paid a ~1us SWDGE desc-gen fixed cost
  per 128 rows (128 Pool instructions -> Pool-bound at ~140us).  Instead we
  use the batched `dma_gather` custom op (one instruction per 1024 rows), at
  the price of int16 indices: indices are bucketed by table chunk of 32768
  rows so chunk-local indices fit in int16, with the chunk base carried by
  the in_ap view.

  Layout: tokens split into 2 halves of 512; each half into 8 subgroups of
  64 tokens.  For each (half h, chunk b) one dma_gather call fetches 1024
  rows = 8 slabs of 128 slots; slab j holds up to 128 (token,k) pairs of
  subgroup j whose table row lies in chunk b (capacity = the mean occupancy,
  128).  Overflow pairs go to a per-half spill region of 4 slabs gathered by
  classic indirect DMA (any chunk, int32 indices).

  Reduction: for each slab, a [128,64] fp32r mask M[slot, j] =
  w[slot] * (tokloc[slot] == j) is built on DVE (is_equal on an iota table,
  then multiply; tokloc/weights are host-prepped per slot).  PE matmuls
  lhsT=rows (fp32r, a free bitcast of the gathered fp32) x rhs=mask
  accumulate yT[d, token] into PSUM; the spill slabs use a 512-wide mask
  over the whole half and accumulate last.  Stage 2 (out_proj) contracts
  yT with w_out.T (fp32r) per 128-token group, adds bias on DVE, DMAs out.

  The dma_gather Q7 ucode reads index i of a call from the idx tile at
  [16 + i%16, i//16] on the NEFF path (queue 0 channel base), while the
  bass-level interpreter reads [i%16, i//16]; the host writes both bands.
"""

import numpy as np

B, T, K = 4, 2048, 16
C, D, E = 262144, 256, 512
NCORES = 8
NTOK = B * T                      # 8192 tokens
TPC = NTOK // NCORES              # 1024 tokens per core
P = 128
HALVES = 2
HTOK = TPC // HALVES              # 512 tokens per half
NB = 8                            # value chunks
CHUNK = C // NB                   # 32768 rows, int16-addressable
NW = 8                            # subgroups per half
WTOK = HTOK // NW                 # 64 tokens per subgroup
NIDX_CALL = NW * P                # 1024 indices per dma_gather call
SPILL_SLABS = 4                   # per half
SPILL_CAP = SPILL_SLABS * P       # 512
MAIN_SLABS = HALVES * NB * NW     # 128
SPILL_TOT = HALVES * SPILL_SLABS  # 8

_CACHE = {}


def _build_bass():
    import concourse.bass as bass
    import concourse.mybir as mybir
    from concourse import bacc, library_config
    from concourse.tile import TileContext

    fp32 = mybir.dt.float32
    f32r = mybir.dt.float32r
    i16 = mybir.dt.int16
    i32 = mybir.dt.int32
    eq = mybir.AluOpType.is_equal
    mul = mybir.AluOpType.mult
    nc = bacc.Bacc(
        "TRN2", target_bir_lowering=False, debug=False, num_devices=NCORES
    )

    kb = nc.dram_tensor("kb", [C, D], f32r, kind="ExternalInput")
    idx16 = nc.dram_tensor("idx16", [P, HALVES * NB * (NIDX_CALL // 16)], i16,
                           kind="ExternalInput")
    idxsp = nc.dram_tensor("idxsp", [P, SPILL_TOT], i32, kind="ExternalInput")
    wslot = nc.dram_tensor("wslot", [P, MAIN_SLABS], fp32, kind="ExternalInput")
    tokloc = nc.dram_tensor("tokloc", [P, MAIN_SLABS], i16, kind="ExternalInput")
    wsp = nc.dram_tensor("wsp", [P, SPILL_TOT], fp32, kind="ExternalInput")
    toksp = nc.dram_tensor("toksp", [P, SPILL_TOT], i16, kind="ExternalInput")
    iota64 = nc.dram_tensor("iota64", [P, WTOK], i16, kind="ExternalInput")
    iota512 = nc.dram_tensor("iota512", [P, HTOK], i16, kind="ExternalInput")
    wout = nc.dram_tensor("wout", [P, 2 * E], f32r, kind="ExternalInput")
    bias = nc.dram_tensor("bias", [1, E], f32r, kind="ExternalInput")
    ones = nc.dram_tensor("ones", [1, P], f32r, kind="ExternalInput")
    out = nc.dram_tensor("out", [TPC, E], fp32, kind="ExternalOutput")

    COLS = NIDX_CALL // 16  # idx16 columns per call

    with TileContext(nc) as tc:
        with (
            tc.tile_pool(name="const", bufs=1) as cpool,
            tc.tile_pool(name="gath", bufs=10) as gpool,
            tc.tile_pool(name="mask", bufs=4) as mpool,
            tc.tile_pool(name="spill", bufs=2) as sppool,
            tc.tile_pool(name="spmask", bufs=2) as smpool,
            tc.tile_pool(name="y", bufs=2) as ypool,
            tc.tile_pool(name="o", bufs=8) as opool,
            tc.tile_pool(name="psy", bufs=2, space="PSUM") as psy,
            tc.tile_pool(name="pso", bufs=4, space="PSUM") as pso,
        ):
            nc.gpsimd.load_library(library_config.mlp)

            idx_sb = cpool.tile([P, HALVES * NB * COLS], i16)
            nc.sync.dma_start(out=idx_sb[:], in_=idx16[:, :])
            idxsp_sb = cpool.tile([P, SPILL_TOT], i32)
            nc.sync.dma_start(out=idxsp_sb[:], in_=idxsp[:, :])
            w_sb = cpool.tile([P, MAIN_SLABS], fp32)
            nc.sync.dma_start(out=w_sb[:], in_=wslot[:, :])
            tl_sb = cpool.tile([P, MAIN_SLABS], i16)
            nc.sync.dma_start(out=tl_sb[:], in_=tokloc[:, :])
            wsp_sb = cpool.tile([P, SPILL_TOT], fp32)
            nc.sync.dma_start(out=wsp_sb[:], in_=wsp[:, :])
            tsp_sb = cpool.tile([P, SPILL_TOT], i16)
            nc.sync.dma_start(out=tsp_sb[:], in_=toksp[:, :])
            io64_sb = cpool.tile([P, WTOK], i16)
            nc.sync.dma_start(out=io64_sb[:], in_=iota64[:, :])
            io512_sb = cpool.tile([P, HTOK], i16)
            nc.sync.dma_start(out=io512_sb[:], in_=iota512[:, :])
            wo_sb = cpool.tile([P, 2 * E], f32r)
            nc.sync.dma_start(out=wo_sb[:], in_=wout[:, :])
            b_sb = cpool.tile([1, E], f32r)
            nc.sync.dma_start(out=b_sb[:], in_=bias[:, :])
            one_sb = cpool.tile([1, P], f32r)
            nc.sync.dma_start(out=one_sb[:], in_=ones[:, :])

            for h in range(HALVES):
                yt = psy.tile([P, 2 * HTOK], fp32, tag="yt")
                # start=True zeroes the whole 2KB psum zero-region, which
                # would wipe earlier 64-col writes in the same bank: zero the
                # banks once and accumulate-only (start=False everywhere).
                nc.vector.memset(yt[:], 0.0)

                gs = []
                for b in range(NB):
                    g = gpool.tile([P, NW, D], f32r, tag="g")
                    col0 = (h * NB + b) * COLS
                    nc.gpsimd.dma_gather(
                        out_ap=g[:],
                        in_ap=kb[b * CHUNK:(b + 1) * CHUNK, :],
                        idxs_ap=idx_sb[:, col0:col0 + COLS],
                        num_idxs=NIDX_CALL,
                        num_idxs_reg=NIDX_CALL,
                        elem_size=D,
                    )
                    gs.append(g)

                if h == 0:
                    # issue BOTH halves' spill gathers now: their desc-gen
                    # overlaps h0's transfers and the data arrives well before
                    # each half's epilogue (instead of queueing after all
                    # gathers and serializing the tail).
                    sp_tiles = []
                    for hh in range(HALVES):
                        sp = sppool.tile([P, SPILL_SLABS, D], f32r, tag="sp")
                        for s in range(SPILL_SLABS):
                            col = hh * SPILL_SLABS + s
                            nc.gpsimd.indirect_dma_start(
                                out=sp[:, s, :],
                                out_offset=None,
                                in_=kb[:, :],
                                in_offset=bass.IndirectOffsetOnAxis(
                                    ap=idxsp_sb[:, col:col + 1], axis=0
                                ),
                            )
                        sp_tiles.append(sp)
                sp = sp_tiles[h]

                msp = smpool.tile([P, SPILL_SLABS, HTOK], f32r, tag="msp")
                sblk = h * SPILL_SLABS
                nc.vector.tensor_tensor(
                    out=msp[:],
                    in0=io512_sb[:].unsqueeze(1)
                        .broadcast_to([P, SPILL_SLABS, HTOK]),
                    in1=tsp_sb[:, sblk:sblk + SPILL_SLABS].unsqueeze(2)
                        .broadcast_to([P, SPILL_SLABS, HTOK]),
                    op=eq,
                )
                nc.vector.tensor_tensor(
                    out=msp[:],
                    in0=msp[:],
                    in1=wsp_sb[:, sblk:sblk + SPILL_SLABS].unsqueeze(2)
                        .broadcast_to([P, SPILL_SLABS, HTOK]),
                    op=mul,
                )
                for s in range(SPILL_SLABS):
                    for ch in range(2):
                        nc.tensor.matmul(
                            out=yt[:, ch * HTOK:(ch + 1) * HTOK],
                            lhsT=sp[:, s, ch * P:(ch + 1) * P],
                            rhs=msp[:, s, :],
                            start=False,
                            stop=False,
                            skip_group_check=True,
                        )

                # mask-matmul reduction, bucket by bucket
                for b in range(NB):
                    blk = (h * NB + b) * NW
                    mask = mpool.tile([P, NW, WTOK], f32r, tag="m")
                    nc.vector.tensor_tensor(
                        out=mask[:],
                        in0=io64_sb[:].unsqueeze(1).broadcast_to([P, NW, WTOK]),
                        in1=tl_sb[:, blk:blk + NW].unsqueeze(2)
                            .broadcast_to([P, NW, WTOK]),
                        op=eq,
                    )
                    nc.vector.tensor_tensor(
                        out=mask[:],
                        in0=mask[:],
                        in1=w_sb[:, blk:blk + NW].unsqueeze(2)
                            .broadcast_to([P, NW, WTOK]),
                        op=mul,
                    )
                    for j in range(NW):
                        for ch in range(2):
                            nc.tensor.matmul(
                                out=yt[:, ch * HTOK + j * WTOK:
                                       ch * HTOK + (j + 1) * WTOK],
                                lhsT=gs[b][:, j, ch * P:(ch + 1) * P],
                                rhs=mask[:, j, :],
                                start=False,
                                stop=(b == NB - 1 and j == NW - 1),
                                skip_group_check=True,
                            )

                yb = ypool.tile([P, 2 * HTOK], f32r, tag="yb")
                for g4 in range(HTOK // P):
                    for ch in range(2):
                        nc.vector.tensor_copy(
                            out=yb[:, ch * HTOK + g4 * P:ch * HTOK + (g4 + 1) * P],
                            in_=yt[:, ch * HTOK + g4 * P:ch * HTOK + (g4 + 1) * P],
                        )

                for g4 in range(HTOK // P):
                    ops = pso.tile([P, E], fp32, tag="ops")
                    for ch in range(2):
                        nc.tensor.matmul(
                            out=ops[:],
                            lhsT=yb[:, ch * HTOK + g4 * P:
                                    ch * HTOK + (g4 + 1) * P],
                            rhs=wo_sb[:, ch * E:(ch + 1) * E],
                            start=(ch == 0),
                            stop=False,
                        )
                    # bias add as a K=1 matmul: ones[1,P]^T x bias[1,E]
                    nc.tensor.matmul(
                        out=ops[:],
                        lhsT=one_sb[:, :],
                        rhs=b_sb[:, :],
                        start=False,
                        stop=True,
                    )
                    osb = opool.tile([P, E], fp32, tag="osb")
                    nc.scalar.copy(out=osb[:], in_=ops[:])
                    row0 = (h * (HTOK // P) + g4) * P
                    nc.sync.dma_start(out=out[row0:row0 + P, :], in_=osb[:])

    nc.compile()
    return nc


def _host_prep(weights, indexes, w_out, b_out):
    """Bucket/sort (token,k) pairs per core and build all device-side arrays."""
    wflat = np.ascontiguousarray(weights, dtype=np.float32).reshape(NTOK, K)
    iflat = np.ascontiguousarray(indexes).reshape(NTOK, K).astype(np.int64)

    woutT = np.ascontiguousarray(w_out, dtype=np.float32).T      # [D, E]
    wout_host = np.ascontiguousarray(
        woutT.reshape(2, P, E).transpose(1, 0, 2).reshape(P, 2 * E)
    )
    bias_host = np.asarray(b_out, dtype=np.float32).reshape(1, E)
    ones_host = np.ones((1, P), dtype=np.float32)
    iota64_h = np.ascontiguousarray(
        np.broadcast_to(np.arange(WTOK, dtype=np.int16), (P, WTOK))
    )
    iota512_h = np.ascontiguousarray(
        np.broadcast_to(np.arange(HTOK, dtype=np.int16), (P, HTOK))
    )

    COLS = NIDX_CALL // 16
    in_maps = []
    for c in range(NCORES):
        ic = iflat[c * TPC:(c + 1) * TPC].ravel()          # [16384]
        wc = wflat[c * TPC:(c + 1) * TPC].ravel()
        t = np.repeat(np.arange(TPC, dtype=np.int64), K)   # token per pair

        h = t // HTOK
        wsub = (t % HTOK) // WTOK
        b = ic // CHUNK
        key = (h * NB + b) * NW + wsub                     # 0..127 slab id

        order = np.argsort(key, kind="stable")
        ks = key[order]
        iv = ic[order]
        wv = wc[order]
        tv = t[order]
        starts = np.searchsorted(ks, np.arange(MAIN_SLABS))
        rank = np.arange(TPC * K) - starts[ks]

        idx16_host = np.zeros((P, HALVES * NB * COLS), np.int16)
        wslot_host = np.zeros((P, MAIN_SLABS), np.float32)
        tokloc_host = np.zeros((P, MAIN_SLABS), np.int16)
        idxsp_host = np.zeros((P, SPILL_TOT), np.int32)
        wsp_host = np.zeros((P, SPILL_TOT), np.float32)
        toksp_host = np.zeros((P, SPILL_TOT), np.int16)

        main = rank < P
        mk, mr = ks[main], rank[main]
        mi, mw, mt = iv[main], wv[main], tv[main]
        mh = mk // (NB * NW)
        mb = (mk // NW) % NB
        mj = mk % NW
        slot = mj * P + mr                                 # slot within call
        col = (mh * NB + mb) * COLS + slot // 16
        idx_local = (mi - mb * CHUNK).astype(np.int16)
        idx16_host[slot % 16, col] = idx_local             # interp layout
        idx16_host[16 + slot % 16, col] = idx_local        # NEFF Q7 layout
        wslot_host[mr, mk] = mw
        tokloc_host[mr, mk] = (mt - (mh * HTOK + mj * WTOK)).astype(np.int16)

        sh = ks[~main] // (NB * NW)                        # spill half
        si, sw, st = iv[~main], wv[~main], tv[~main]
        for hh in range(HALVES):
            sel = sh == hh
            n = int(sel.sum())
            if n > SPILL_CAP:
                raise ValueError(
                    f"spill overflow: core {c} half {hh} needs {n} > {SPILL_CAP}"
                )
            r = np.arange(n)
            idxsp_host[r % P, hh * SPILL_SLABS + r // P] = si[sel]
            wsp_host[r % P, hh * SPILL_SLABS + r // P] = sw[sel]
            toksp_host[r % P, hh * SPILL_SLABS + r // P] = (
                st[sel] - hh * HTOK
            ).astype(np.int16)

        in_maps.append({
            "idx16": idx16_host,
            "idxsp": idxsp_host,
            "wslot": wslot_host,
            "tokloc": tokloc_host,
            "wsp": wsp_host,
            "toksp": toksp_host,
            "iota64": iota64_h,
            "iota512": iota512_h,
            "wout": wout_host,
            "bias": bias_host,
            "ones": ones_host,
        })
    return in_maps


def kernel(weights, indexes, knowledge_base, w_out, b_out):
    from concourse.bass_utils import run_bass_kernel_spmd

    if "nc" not in _CACHE:
        _CACHE["nc"] = _build_bass()
    nc = _CACHE["nc"]

    kb_host = np.ascontiguousarray(knowledge_base, dtype=np.float32)
    in_maps = _host_prep(weights, indexes, w_out, b_out)
    for m in in_maps:
        m["kb"] = kb_host

    res = run_bass_kernel_spmd(nc, in_maps, list(range(NCORES)))
    out = np.concatenate([res.results[c]["out"] for c in range(NCORES)], axis=0)
    return out.reshape(B, T, E).astype(np.float32)


# revision 6
# speedup vs baseline: 1.0992x; 1.0992x over previous
"""Trainium2 Bass kernel for nn_KnowledgeBaseLookup (bucketed dma_gather design).

Computation (see reference):
    lookup = knowledge_base[indexes]            # (B,T,K,D) gather
    y      = einsum('btk,btkd->btd', weights, lookup)
    out    = y @ w_out.T + b_out                # (B,T,E)

Sharding: data-parallel over the B*T token dim across 8 cores; the
knowledge_base table is replicated per core.

Per-core design (1024 tokens, 16384 gathered rows):
  The old per-slab indirect-DMA gather paid a ~1us SWDGE desc-gen fixed cost
  per 128 rows (128 Pool instructions -> Pool-bound at ~140us).  Instead we
  use the batched `dma_gather` custom op (one instruction per 1024 rows), at
  the price of int16 indices: indices are bucketed by table chunk of 32768
  rows so chunk-local indices fit in int16, with the chunk base carried by
  the in_ap view.

  Layout: tokens split into 2 halves of 512; each half into 8 subgroups of
  64 tokens.  For each (half h, chunk b) one dma_gather call fetches 1024
  rows = 8 slabs of 128 slots; slab j holds up to 128 (token,k) pairs of
  subgroup j whose table row lies in chunk b (capacity = the mean occupancy,
  128).  Overflow pairs go to a per-half spill region of 4 slabs gathered by
  classic indirect DMA (any chunk, int32 indices).

  Reduction: for each slab, a [128,64] fp32r mask M[slot, j] =
  w[slot] * (tokloc[slot] == j) is built on DVE (is_equal on an iota table,
  then multiply; tokloc/weights are host-prepped per slot).  PE matmuls
  lhsT=rows (fp32r, a free bitcast of the gathered fp32) x rhs=mask
  accumulate yT[d, token] into PSUM; the spill slabs use a 512-wide mask
  over the whole half and accumulate last.  Stage 2 (out_proj) contracts
  yT with w_out.T (fp32r) per 128-token group, adds bias on DVE, DMAs out.

  The dma_gather Q7 ucode reads index i of a call from the idx tile at
  [16 + i%16, i//16] on the NEFF path (queue 0 channel base), while the
  bass-level interpreter reads [i%16, i//16]; the host writes both bands.
"""

import numpy as np

B, T, K = 4, 2048, 16
C, D, E = 262144, 256, 512
NCORES = 8
NTOK = B * T                      # 8192 tokens
TPC = NTOK // NCORES              # 1024 tokens per core
P = 128
HALVES = 2
HTOK = TPC // HALVES              # 512 tokens per half
NB = 8                            # value chunks
CHUNK = C // NB                   # 32768 rows, int16-addressable
NW = 8                            # subgroups per half
WTOK = HTOK // NW                 # 64 tokens per subgroup
NIDX_CALL = NW * P                # 1024 indices per dma_gather call
SPILL_SLABS = 4                   # per half
SPILL_CAP = SPILL_SLABS * P       # 512
MAIN_SLABS = HALVES * NB * NW     # 128
SPILL_TOT = HALVES * SPILL_SLABS  # 8

_CACHE = {}


def _build_bass():
    import concourse.bass as bass
    import concourse.mybir as mybir
    from concourse import bacc, library_config
    from concourse.tile import TileContext

    fp32 = mybir.dt.float32
    f32r = mybir.dt.float32r
    i16 = mybir.dt.int16
    i32 = mybir.dt.int32
    eq = mybir.AluOpType.is_equal
    mul = mybir.AluOpType.mult
    nc = bacc.Bacc(
        "TRN2", target_bir_lowering=False, debug=False, num_devices=NCORES
    )

    kb = nc.dram_tensor("kb", [C, D], f32r, kind="ExternalInput")
    idx16 = nc.dram_tensor("idx16", [P, HALVES * NB * (NIDX_CALL // 16)], i16,
                           kind="ExternalInput")
    idxsp = nc.dram_tensor("idxsp", [P, SPILL_TOT], i32, kind="ExternalInput")
    wslot = nc.dram_tensor("wslot", [P, MAIN_SLABS], fp32, kind="ExternalInput")
    tokloc = nc.dram_tensor("tokloc", [P, MAIN_SLABS], i16, kind="ExternalInput")
    wsp = nc.dram_tensor("wsp", [P, SPILL_TOT], fp32, kind="ExternalInput")
    toksp = nc.dram_tensor("toksp", [P, SPILL_TOT], i16, kind="ExternalInput")
    iota64 = nc.dram_tensor("iota64", [P, WTOK], i16, kind="ExternalInput")
    iota512 = nc.dram_tensor("iota512", [P, HTOK], i16, kind="ExternalInput")
    wout = nc.dram_tensor("wout", [P, 2 * E], f32r, kind="ExternalInput")
    bias = nc.dram_tensor("bias", [1, E], f32r, kind="ExternalInput")
    ones = nc.dram_tensor("ones", [1, P], f32r, kind="ExternalInput")
    out = nc.dram_tensor("out", [TPC, E], fp32, kind="ExternalOutput")

    COLS = NIDX_CALL // 16  # idx16 columns per call

    with TileContext(nc) as tc:
        with (
            tc.tile_pool(name="const", bufs=1) as cpool,
            tc.tile_pool(name="gath", bufs=10) as gpool,
            tc.tile_pool(name="mask", bufs=4) as mpool,
            tc.tile_pool(name="spill", bufs=2) as sppool,
            tc.tile_pool(name="spmask", bufs=2) as smpool,
            tc.tile_pool(name="y", bufs=2) as ypool,
            tc.tile_pool(name="o", bufs=8) as opool,
            tc.tile_pool(name="psy", bufs=2, space="PSUM") as psy,
            tc.tile_pool(name="pso", bufs=4, space="PSUM") as pso,
        ):
            nc.gpsimd.load_library(library_config.mlp)

            idx_sb = cpool.tile([P, HALVES * NB * COLS], i16)
            nc.sync.dma_start(out=idx_sb[:], in_=idx16[:, :])
            idxsp_sb = cpool.tile([P, SPILL_TOT], i32)
            nc.sync.dma_start(out=idxsp_sb[:], in_=idxsp[:, :])
            w_sb = cpool.tile([P, MAIN_SLABS], fp32)
            nc.sync.dma_start(out=w_sb[:], in_=wslot[:, :])
            tl_sb = cpool.tile([P, MAIN_SLABS], i16)
            nc.sync.dma_start(out=tl_sb[:], in_=tokloc[:, :])
            wsp_sb = cpool.tile([P, SPILL_TOT], fp32)
            nc.sync.dma_start(out=wsp_sb[:], in_=wsp[:, :])
            tsp_sb = cpool.tile([P, SPILL_TOT], i16)
            nc.sync.dma_start(out=tsp_sb[:], in_=toksp[:, :])
            io64_sb = cpool.tile([P, WTOK], i16)
            nc.sync.dma_start(out=io64_sb[:], in_=iota64[:, :])
            io512_sb = cpool.tile([P, HTOK], i16)
            nc.sync.dma_start(out=io512_sb[:], in_=iota512[:, :])
            wo_sb = cpool.tile([P, 2 * E], f32r)
            nc.sync.dma_start(out=wo_sb[:], in_=wout[:, :])
            b_sb = cpool.tile([1, E], f32r)
            nc.sync.dma_start(out=b_sb[:], in_=bias[:, :])
            one_sb = cpool.tile([1, P], f32r)
            nc.sync.dma_start(out=one_sb[:], in_=ones[:, :])

            for h in range(HALVES):
                yt = psy.tile([P, 2 * HTOK], fp32, tag="yt")
                # start=True zeroes the whole 2KB psum zero-region, which
                # would wipe earlier 64-col writes in the same bank: zero the
                # banks once and accumulate-only (start=False everywhere).
                nc.vector.memset(yt[:], 0.0)

                gs = []
                for b in range(NB):
                    g = gpool.tile([P, NW, D], f32r, tag="g")
                    col0 = (h * NB + b) * COLS
                    nc.gpsimd.dma_gather(
                        out_ap=g[:],
                        in_ap=kb[b * CHUNK:(b + 1) * CHUNK, :],
                        idxs_ap=idx_sb[:, col0:col0 + COLS],
                        num_idxs=NIDX_CALL,
                        num_idxs_reg=NIDX_CALL,
                        elem_size=D,
                    )
                    gs.append(g)

                if h == 0:
                    # issue BOTH halves' spill gathers now: their desc-gen
                    # overlaps h0's transfers and the data arrives well before
                    # each half's epilogue (instead of queueing after all
                    # gathers and serializing the tail).
                    sp_tiles = []
                    for hh in range(HALVES):
                        sp = sppool.tile([P, SPILL_SLABS, D], f32r, tag="sp")
                        for s in range(SPILL_SLABS):
                            col = hh * SPILL_SLABS + s
                            nc.gpsimd.indirect_dma_start(
                                out=sp[:, s, :],
                                out_offset=None,
                                in_=kb[:, :],
                                in_offset=bass.IndirectOffsetOnAxis(
                                    ap=idxsp_sb[:, col:col + 1], axis=0
                                ),
                            )
                        sp_tiles.append(sp)
                sp = sp_tiles[h]

                def do_spill(last):
                    msp = smpool.tile([P, SPILL_SLABS, HTOK], f32r, tag="msp")
                    sblk = h * SPILL_SLABS
                    nc.vector.tensor_tensor(
                        out=msp[:],
                        in0=io512_sb[:].unsqueeze(1)
                            .broadcast_to([P, SPILL_SLABS, HTOK]),
                        in1=tsp_sb[:, sblk:sblk + SPILL_SLABS].unsqueeze(2)
                            .broadcast_to([P, SPILL_SLABS, HTOK]),
                        op=eq,
                    )
                    nc.vector.tensor_tensor(
                        out=msp[:],
                        in0=msp[:],
                        in1=wsp_sb[:, sblk:sblk + SPILL_SLABS].unsqueeze(2)
                            .broadcast_to([P, SPILL_SLABS, HTOK]),
                        op=mul,
                    )
                    for s in range(SPILL_SLABS):
                        for ch in range(2):
                            nc.tensor.matmul(
                                out=yt[:, ch * HTOK:(ch + 1) * HTOK],
                                lhsT=sp[:, s, ch * P:(ch + 1) * P],
                                rhs=msp[:, s, :],
                                start=False,
                                stop=(last and s == SPILL_SLABS - 1),
                                skip_group_check=True,
                            )

                if h == 1:
                    # h1 spill data lands long before its last gathers:
                    # run its matmuls first so only bucket 7 is in the tail.
                    do_spill(last=False)

                # mask-matmul reduction, bucket by bucket
                for b in range(NB):
                    blk = (h * NB + b) * NW
                    mask = mpool.tile([P, NW, WTOK], f32r, tag="m")
                    nc.vector.tensor_tensor(
                        out=mask[:],
                        in0=io64_sb[:].unsqueeze(1).broadcast_to([P, NW, WTOK]),
                        in1=tl_sb[:, blk:blk + NW].unsqueeze(2)
                            .broadcast_to([P, NW, WTOK]),
                        op=eq,
                    )
                    nc.vector.tensor_tensor(
                        out=mask[:],
                        in0=mask[:],
                        in1=w_sb[:, blk:blk + NW].unsqueeze(2)
                            .broadcast_to([P, NW, WTOK]),
                        op=mul,
                    )
                    for j in range(NW):
                        for ch in range(2):
                            nc.tensor.matmul(
                                out=yt[:, ch * HTOK + j * WTOK:
                                       ch * HTOK + (j + 1) * WTOK],
                                lhsT=gs[b][:, j, ch * P:(ch + 1) * P],
                                rhs=mask[:, j, :],
                                start=False,
                                stop=(h == 1 and b == NB - 1 and j == NW - 1),
                                skip_group_check=True,
                            )

                if h == 0:
                    do_spill(last=True)

                yb = ypool.tile([P, 2 * HTOK], f32r, tag="yb")
                for g4 in range(HTOK // P):
                    for ch in range(2):
                        nc.vector.tensor_copy(
                            out=yb[:, ch * HTOK + g4 * P:ch * HTOK + (g4 + 1) * P],
                            in_=yt[:, ch * HTOK + g4 * P:ch * HTOK + (g4 + 1) * P],
                        )

                for g4 in range(HTOK // P):
                    ops = pso.tile([P, E], fp32, tag="ops")
                    for ch in range(2):
                        nc.tensor.matmul(
                            out=ops[:],
                            lhsT=yb[:, ch * HTOK + g4 * P:
                                    ch * HTOK + (g4 + 1) * P],
                            rhs=wo_sb[:, ch * E:(ch + 1) * E],
                            start=(ch == 0),
                            stop=False,
                        )
                    # bias add as a K=1 matmul: ones[1,P]^T x bias[1,E]
                    nc.tensor.matmul(
                        out=ops[:],
                        lhsT=one_sb[:, :],
                        rhs=b_sb[:, :],
                        start=False,
                        stop=True,
                    )
                    osb = opool.tile([P, E], fp32, tag="osb")
                    nc.scalar.copy(out=osb[:], in_=ops[:])
                    row0 = (h * (HTOK // P) + g4) * P
                    nc.sync.dma_start(out=out[row0:row0 + P, :], in_=osb[:])

    nc.compile()
    return nc


def _host_prep(weights, indexes, w_out, b_out):
    """Bucket/sort (token,k) pairs per core and build all device-side arrays."""
    wflat = np.ascontiguousarray(weights, dtype=np.float32).reshape(NTOK, K)
    iflat = np.ascontiguousarray(indexes).reshape(NTOK, K).astype(np.int64)

    woutT = np.ascontiguousarray(w_out, dtype=np.float32).T      # [D, E]
    wout_host = np.ascontiguousarray(
        woutT.reshape(2, P, E).transpose(1, 0, 2).reshape(P, 2 * E)
    )
    bias_host = np.asarray(b_out, dtype=np.float32).reshape(1, E)
    ones_host = np.ones((1, P), dtype=np.float32)
    iota64_h = np.ascontiguousarray(
        np.broadcast_to(np.arange(WTOK, dtype=np.int16), (P, WTOK))
    )
    iota512_h = np.ascontiguousarray(
        np.broadcast_to(np.arange(HTOK, dtype=np.int16), (P, HTOK))
    )

    COLS = NIDX_CALL // 16
    in_maps = []
    for c in range(NCORES):
        ic = iflat[c * TPC:(c + 1) * TPC].ravel()          # [16384]
        wc = wflat[c * TPC:(c + 1) * TPC].ravel()
        t = np.repeat(np.arange(TPC, dtype=np.int64), K)   # token per pair

        h = t // HTOK
        wsub = (t % HTOK) // WTOK
        b = ic // CHUNK
        key = (h * NB + b) * NW + wsub                     # 0..127 slab id

        order = np.argsort(key, kind="stable")
        ks = key[order]
        iv = ic[order]
        wv = wc[order]
        tv = t[order]
        starts = np.searchsorted(ks, np.arange(MAIN_SLABS))
        rank = np.arange(TPC * K) - starts[ks]

        idx16_host = np.zeros((P, HALVES * NB * COLS), np.int16)
        wslot_host = np.zeros((P, MAIN_SLABS), np.float32)
        tokloc_host = np.zeros((P, MAIN_SLABS), np.int16)
        idxsp_host = np.zeros((P, SPILL_TOT), np.int32)
        wsp_host = np.zeros((P, SPILL_TOT), np.float32)
        toksp_host = np.zeros((P, SPILL_TOT), np.int16)

        main = rank < P
        mk, mr = ks[main], rank[main]
        mi, mw, mt = iv[main], wv[main], tv[main]
        mh = mk // (NB * NW)
        mb = (mk // NW) % NB
        mj = mk % NW
        slot = mj * P + mr                                 # slot within call
        col = (mh * NB + mb) * COLS + slot // 16
        idx_local = (mi - mb * CHUNK).astype(np.int16)
        idx16_host[slot % 16, col] = idx_local             # interp layout
        idx16_host[16 + slot % 16, col] = idx_local        # NEFF Q7 layout
        wslot_host[mr, mk] = mw
        tokloc_host[mr, mk] = (mt - (mh * HTOK + mj * WTOK)).astype(np.int16)

        sh = ks[~main] // (NB * NW)                        # spill half
        si, sw, st = iv[~main], wv[~main], tv[~main]
        for hh in range(HALVES):
            sel = sh == hh
            n = int(sel.sum())
            if n > SPILL_CAP:
                raise ValueError(
                    f"spill overflow: core {c} half {hh} needs {n} > {SPILL_CAP}"
                )
            r = np.arange(n)
            idxsp_host[r % P, hh * SPILL_SLABS + r // P] = si[sel]
            wsp_host[r % P, hh * SPILL_SLABS + r // P] = sw[sel]
            toksp_host[r % P, hh * SPILL_SLABS + r // P] = (
                st[sel] - hh * HTOK
            ).astype(np.int16)

        in_maps.append({
            "idx16": idx16_host,
            "idxsp": idxsp_host,
            "wslot": wslot_host,
            "tokloc": tokloc_host,
            "wsp": wsp_host,
            "toksp": toksp_host,
            "iota64": iota64_h,
            "iota512": iota512_h,
            "wout": wout_host,
            "bias": bias_host,
            "ones": ones_host,
        })
    return in_maps


def kernel(weights, indexes, knowledge_base, w_out, b_out):
    from concourse.bass_utils import run_bass_kernel_spmd

    if "nc" not in _CACHE:
        _CACHE["nc"] = _build_bass()
    nc = _CACHE["nc"]

    kb_host = np.ascontiguousarray(knowledge_base, dtype=np.float32)
    in_maps = _host_prep(weights, indexes, w_out, b_out)
    for m in in_maps:
        m["kb"] = kb_host

    res = run_bass_kernel_spmd(nc, in_maps, list(range(NCORES)))
    out = np.concatenate([res.results[c]["out"] for c in range(NCORES)], axis=0)
    return out.reshape(B, T, E).astype(np.float32)


# revision 7
# speedup vs baseline: 1.1196x; 1.0185x over previous
"""Trainium2 Bass kernel for nn_KnowledgeBaseLookup (bucketed dma_gather design).

Computation (see reference):
    lookup = knowledge_base[indexes]            # (B,T,K,D) gather
    y      = einsum('btk,btkd->btd', weights, lookup)
    out    = y @ w_out.T + b_out                # (B,T,E)

Sharding: data-parallel over the B*T token dim across 8 cores; the
knowledge_base table is replicated per core.

Per-core design (1024 tokens, 16384 gathered rows):
  The old per-slab indirect-DMA gather paid a ~1us SWDGE desc-gen fixed cost
  per 128 rows (128 Pool instructions -> Pool-bound at ~140us).  Instead we
  use the batched `dma_gather` custom op (one instruction per 1024 rows), at
  the price of int16 indices: indices are bucketed by table chunk of 32768
  rows so chunk-local indices fit in int16, with the chunk base carried by
  the in_ap view.

  Layout: tokens split into 2 halves of 512; each half into 8 subgroups of
  64 tokens.  For each (half h, chunk b) one dma_gather call fetches 1024
  rows = 8 slabs of 128 slots; slab j holds up to 128 (token,k) pairs of
  subgroup j whose table row lies in chunk b (capacity = the mean occupancy,
  128).  Overflow pairs go to a per-half spill region of 4 slabs gathered by
  classic indirect DMA (any chunk, int32 indices).

  Reduction: for each slab, a [128,64] fp32r mask M[slot, j] =
  w[slot] * (tokloc[slot] == j) is built on DVE (is_equal on an iota table,
  then multiply; tokloc/weights are host-prepped per slot).  PE matmuls
  lhsT=rows (fp32r, a free bitcast of the gathered fp32) x rhs=mask
  accumulate yT[d, token] into PSUM; the spill slabs use a 512-wide mask
  over the whole half and accumulate last.  Stage 2 (out_proj) contracts
  yT with w_out.T (fp32r) per 128-token group, adds bias on DVE, DMAs out.

  The dma_gather Q7 ucode reads index i of a call from the idx tile at
  [16 + i%16, i//16] on the NEFF path (queue 0 channel base), while the
  bass-level interpreter reads [i%16, i//16]; the host writes both bands.
"""

import numpy as np

B, T, K = 4, 2048, 16
C, D, E = 262144, 256, 512
NCORES = 8
NTOK = B * T                      # 8192 tokens
TPC = NTOK // NCORES              # 1024 tokens per core
P = 128
HALVES = 2
HTOK = TPC // HALVES              # 512 tokens per half
NB = 8                            # value chunks
CHUNK = C // NB                   # 32768 rows, int16-addressable
NW = 8                            # subgroups per half
WTOK = HTOK // NW                 # 64 tokens per subgroup
NIDX_CALL = NW * P                # 1024 indices per dma_gather call
SPILL_SLABS = 4                   # per half
SPILL_CAP = SPILL_SLABS * P       # 512
MAIN_SLABS = HALVES * NB * NW     # 128
SPILL_TOT = HALVES * SPILL_SLABS  # 8

_CACHE = {}


def _build_bass():
    import concourse.bass as bass
    import concourse.mybir as mybir
    from concourse import bacc, library_config
    from concourse.tile import TileContext

    fp32 = mybir.dt.float32
    f32r = mybir.dt.float32r
    bf16 = mybir.dt.bfloat16
    i16 = mybir.dt.int16
    i32 = mybir.dt.int32
    eq = mybir.AluOpType.is_equal
    mul = mybir.AluOpType.mult
    nc = bacc.Bacc(
        "TRN2", target_bir_lowering=False, debug=False, num_devices=NCORES
    )

    kb = nc.dram_tensor("kb", [C, D], f32r, kind="ExternalInput")
    idx16 = nc.dram_tensor("idx16", [P, HALVES * NB * (NIDX_CALL // 16)], i16,
                           kind="ExternalInput")
    idxsp = nc.dram_tensor("idxsp", [P, SPILL_TOT], i32, kind="ExternalInput")
    wslot = nc.dram_tensor("wslot", [P, MAIN_SLABS], fp32, kind="ExternalInput")
    tokloc = nc.dram_tensor("tokloc", [P, MAIN_SLABS], i16, kind="ExternalInput")
    wsp = nc.dram_tensor("wsp", [P, SPILL_TOT], fp32, kind="ExternalInput")
    toksp = nc.dram_tensor("toksp", [P, SPILL_TOT], i16, kind="ExternalInput")
    iota64 = nc.dram_tensor("iota64", [P, WTOK], i16, kind="ExternalInput")
    iota512 = nc.dram_tensor("iota512", [P, HTOK], i16, kind="ExternalInput")
    wout = nc.dram_tensor("wout", [P, 2 * E], bf16, kind="ExternalInput")
    bias = nc.dram_tensor("bias", [1, E], bf16, kind="ExternalInput")
    ones = nc.dram_tensor("ones", [1, P], bf16, kind="ExternalInput")
    out = nc.dram_tensor("out", [TPC, E], fp32, kind="ExternalOutput")

    COLS = NIDX_CALL // 16  # idx16 columns per call

    with TileContext(nc) as tc:
        with (
            tc.tile_pool(name="const", bufs=1) as cpool,
            tc.tile_pool(name="gath", bufs=10) as gpool,
            tc.tile_pool(name="mask", bufs=4) as mpool,
            tc.tile_pool(name="spill", bufs=2) as sppool,
            tc.tile_pool(name="spmask", bufs=2) as smpool,
            tc.tile_pool(name="y", bufs=2) as ypool,
            tc.tile_pool(name="o", bufs=8) as opool,
            tc.tile_pool(name="psy", bufs=2, space="PSUM") as psy,
            tc.tile_pool(name="pso", bufs=4, space="PSUM") as pso,
        ):
            nc.gpsimd.load_library(library_config.mlp)

            idx_sb = cpool.tile([P, HALVES * NB * COLS], i16)
            nc.sync.dma_start(out=idx_sb[:], in_=idx16[:, :])
            idxsp_sb = cpool.tile([P, SPILL_TOT], i32)
            nc.sync.dma_start(out=idxsp_sb[:], in_=idxsp[:, :])
            w_sb = cpool.tile([P, MAIN_SLABS], fp32)
            nc.sync.dma_start(out=w_sb[:], in_=wslot[:, :])
            tl_sb = cpool.tile([P, MAIN_SLABS], i16)
            nc.sync.dma_start(out=tl_sb[:], in_=tokloc[:, :])
            wsp_sb = cpool.tile([P, SPILL_TOT], fp32)
            nc.sync.dma_start(out=wsp_sb[:], in_=wsp[:, :])
            tsp_sb = cpool.tile([P, SPILL_TOT], i16)
            nc.sync.dma_start(out=tsp_sb[:], in_=toksp[:, :])
            io64_sb = cpool.tile([P, WTOK], i16)
            nc.sync.dma_start(out=io64_sb[:], in_=iota64[:, :])
            io512_sb = cpool.tile([P, HTOK], i16)
            nc.sync.dma_start(out=io512_sb[:], in_=iota512[:, :])
            wo_sb = cpool.tile([P, 2 * E], bf16)
            nc.sync.dma_start(out=wo_sb[:], in_=wout[:, :])
            b_sb = cpool.tile([1, E], bf16)
            nc.sync.dma_start(out=b_sb[:], in_=bias[:, :])
            one_sb = cpool.tile([1, P], bf16)
            nc.sync.dma_start(out=one_sb[:], in_=ones[:, :])

            for h in range(HALVES):
                yt = psy.tile([P, 2 * HTOK], fp32, tag="yt")
                # start=True zeroes the whole 2KB psum zero-region, which
                # would wipe earlier 64-col writes in the same bank: zero the
                # banks once and accumulate-only (start=False everywhere).
                nc.vector.memset(yt[:], 0.0)

                gs = []
                for b in range(NB):
                    g = gpool.tile([P, NW, D], f32r, tag="g")
                    col0 = (h * NB + b) * COLS
                    if h == 1 and b == NB - 1:
                        # split the final gather so the tail only waits on the
                        # last 2 slabs' worth of matmuls
                        nc.gpsimd.dma_gather(
                            out_ap=g[:, 0:6, :],
                            in_ap=kb[b * CHUNK:(b + 1) * CHUNK, :],
                            idxs_ap=idx_sb[:, col0:col0 + 48],
                            num_idxs=768,
                            num_idxs_reg=768,
                            elem_size=D,
                        )
                        nc.gpsimd.dma_gather(
                            out_ap=g[:, 6:8, :],
                            in_ap=kb[b * CHUNK:(b + 1) * CHUNK, :],
                            idxs_ap=idx_sb[:, col0 + 48:col0 + COLS],
                            num_idxs=256,
                            num_idxs_reg=256,
                            elem_size=D,
                        )
                    else:
                        nc.gpsimd.dma_gather(
                            out_ap=g[:],
                            in_ap=kb[b * CHUNK:(b + 1) * CHUNK, :],
                            idxs_ap=idx_sb[:, col0:col0 + COLS],
                            num_idxs=NIDX_CALL,
                            num_idxs_reg=NIDX_CALL,
                            elem_size=D,
                        )
                    gs.append(g)

                if h == 0:
                    # issue BOTH halves' spill gathers now: their desc-gen
                    # overlaps h0's transfers and the data arrives well before
                    # each half's epilogue (instead of queueing after all
                    # gathers and serializing the tail).
                    sp_tiles = []
                    for hh in range(HALVES):
                        sp = sppool.tile([P, SPILL_SLABS, D], f32r, tag="sp")
                        for s in range(SPILL_SLABS):
                            col = hh * SPILL_SLABS + s
                            nc.gpsimd.indirect_dma_start(
                                out=sp[:, s, :],
                                out_offset=None,
                                in_=kb[:, :],
                                in_offset=bass.IndirectOffsetOnAxis(
                                    ap=idxsp_sb[:, col:col + 1], axis=0
                                ),
                            )
                        sp_tiles.append(sp)
                sp = sp_tiles[h]

                def do_spill(last):
                    msp = smpool.tile([P, SPILL_SLABS, HTOK], f32r, tag="msp")
                    sblk = h * SPILL_SLABS
                    nc.vector.tensor_tensor(
                        out=msp[:],
                        in0=io512_sb[:].unsqueeze(1)
                            .broadcast_to([P, SPILL_SLABS, HTOK]),
                        in1=tsp_sb[:, sblk:sblk + SPILL_SLABS].unsqueeze(2)
                            .broadcast_to([P, SPILL_SLABS, HTOK]),
                        op=eq,
                    )
                    nc.vector.tensor_tensor(
                        out=msp[:],
                        in0=msp[:],
                        in1=wsp_sb[:, sblk:sblk + SPILL_SLABS].unsqueeze(2)
                            .broadcast_to([P, SPILL_SLABS, HTOK]),
                        op=mul,
                    )
                    for s in range(SPILL_SLABS):
                        for ch in range(2):
                            nc.tensor.matmul(
                                out=yt[:, ch * HTOK:(ch + 1) * HTOK],
                                lhsT=sp[:, s, ch * P:(ch + 1) * P],
                                rhs=msp[:, s, :],
                                start=False,
                                stop=(last and s == SPILL_SLABS - 1),
                                skip_group_check=True,
                            )

                if h == 1:
                    # h1 spill data lands long before its last gathers:
                    # run its matmuls first so only bucket 7 is in the tail.
                    do_spill(last=False)

                # mask-matmul reduction, bucket by bucket
                for b in range(NB):
                    blk = (h * NB + b) * NW
                    mask = mpool.tile([P, NW, WTOK], f32r, tag="m")
                    nc.vector.tensor_tensor(
                        out=mask[:],
                        in0=io64_sb[:].unsqueeze(1).broadcast_to([P, NW, WTOK]),
                        in1=tl_sb[:, blk:blk + NW].unsqueeze(2)
                            .broadcast_to([P, NW, WTOK]),
                        op=eq,
                    )
                    nc.vector.tensor_tensor(
                        out=mask[:],
                        in0=mask[:],
                        in1=w_sb[:, blk:blk + NW].unsqueeze(2)
                            .broadcast_to([P, NW, WTOK]),
                        op=mul,
                    )
                    for j in range(NW):
                        for ch in range(2):
                            nc.tensor.matmul(
                                out=yt[:, ch * HTOK + j * WTOK:
                                       ch * HTOK + (j + 1) * WTOK],
                                lhsT=gs[b][:, j, ch * P:(ch + 1) * P],
                                rhs=mask[:, j, :],
                                start=False,
                                stop=(h == 1 and b == NB - 1 and j == NW - 1),
                                skip_group_check=True,
                            )

                if h == 0:
                    do_spill(last=True)

                yb = ypool.tile([P, 2 * HTOK], bf16, tag="yb")
                for g4 in range(HTOK // P):
                    for ch in range(2):
                        nc.vector.tensor_copy(
                            out=yb[:, ch * HTOK + g4 * P:ch * HTOK + (g4 + 1) * P],
                            in_=yt[:, ch * HTOK + g4 * P:ch * HTOK + (g4 + 1) * P],
                        )

                for g4 in range(HTOK // P):
                    ops = pso.tile([P, E], fp32, tag="ops")
                    for ch in range(2):
                        nc.tensor.matmul(
                            out=ops[:],
                            lhsT=yb[:, ch * HTOK + g4 * P:
                                    ch * HTOK + (g4 + 1) * P],
                            rhs=wo_sb[:, ch * E:(ch + 1) * E],
                            start=(ch == 0),
                            stop=False,
                        )
                    # bias add as a K=1 matmul: ones[1,P]^T x bias[1,E]
                    nc.tensor.matmul(
                        out=ops[:],
                        lhsT=one_sb[:, :],
                        rhs=b_sb[:, :],
                        start=False,
                        stop=True,
                    )
                    osb = opool.tile([P, E], fp32, tag="osb")
                    nc.scalar.copy(out=osb[:], in_=ops[:])
                    row0 = (h * (HTOK // P) + g4) * P
                    nc.sync.dma_start(out=out[row0:row0 + P, :], in_=osb[:])

    nc.compile()
    return nc


def _host_prep(weights, indexes, w_out, b_out):
    """Bucket/sort (token,k) pairs per core and build all device-side arrays."""
    wflat = np.ascontiguousarray(weights, dtype=np.float32).reshape(NTOK, K)
    iflat = np.ascontiguousarray(indexes).reshape(NTOK, K).astype(np.int64)

    import ml_dtypes
    woutT = np.ascontiguousarray(w_out, dtype=np.float32).T      # [D, E]
    wout_host = np.ascontiguousarray(
        woutT.reshape(2, P, E).transpose(1, 0, 2).reshape(P, 2 * E)
    ).astype(ml_dtypes.bfloat16)
    bias_host = np.asarray(b_out, dtype=np.float32).reshape(1, E).astype(ml_dtypes.bfloat16)
    ones_host = np.ones((1, P), dtype=ml_dtypes.bfloat16)
    iota64_h = np.ascontiguousarray(
        np.broadcast_to(np.arange(WTOK, dtype=np.int16), (P, WTOK))
    )
    iota512_h = np.ascontiguousarray(
        np.broadcast_to(np.arange(HTOK, dtype=np.int16), (P, HTOK))
    )

    COLS = NIDX_CALL // 16
    in_maps = []
    for c in range(NCORES):
        ic = iflat[c * TPC:(c + 1) * TPC].ravel()          # [16384]
        wc = wflat[c * TPC:(c + 1) * TPC].ravel()
        t = np.repeat(np.arange(TPC, dtype=np.int64), K)   # token per pair

        h = t // HTOK
        wsub = (t % HTOK) // WTOK
        b = ic // CHUNK
        key = (h * NB + b) * NW + wsub                     # 0..127 slab id

        order = np.argsort(key, kind="stable")
        ks = key[order]
        iv = ic[order]
        wv = wc[order]
        tv = t[order]
        starts = np.searchsorted(ks, np.arange(MAIN_SLABS))
        rank = np.arange(TPC * K) - starts[ks]

        idx16_host = np.zeros((P, HALVES * NB * COLS), np.int16)
        wslot_host = np.zeros((P, MAIN_SLABS), np.float32)
        tokloc_host = np.zeros((P, MAIN_SLABS), np.int16)
        idxsp_host = np.zeros((P, SPILL_TOT), np.int32)
        wsp_host = np.zeros((P, SPILL_TOT), np.float32)
        toksp_host = np.zeros((P, SPILL_TOT), np.int16)

        main = rank < P
        mk, mr = ks[main], rank[main]
        mi, mw, mt = iv[main], wv[main], tv[main]
        mh = mk // (NB * NW)
        mb = (mk // NW) % NB
        mj = mk % NW
        slot = mj * P + mr                                 # slot within call
        col = (mh * NB + mb) * COLS + slot // 16
        idx_local = (mi - mb * CHUNK).astype(np.int16)
        idx16_host[slot % 16, col] = idx_local             # interp layout
        idx16_host[16 + slot % 16, col] = idx_local        # NEFF Q7 layout
        wslot_host[mr, mk] = mw
        tokloc_host[mr, mk] = (mt - (mh * HTOK + mj * WTOK)).astype(np.int16)

        sh = ks[~main] // (NB * NW)                        # spill half
        si, sw, st = iv[~main], wv[~main], tv[~main]
        for hh in range(HALVES):
            sel = sh == hh
            n = int(sel.sum())
            if n > SPILL_CAP:
                raise ValueError(
                    f"spill overflow: core {c} half {hh} needs {n} > {SPILL_CAP}"
                )
            r = np.arange(n)
            idxsp_host[r % P, hh * SPILL_SLABS + r // P] = si[sel]
            wsp_host[r % P, hh * SPILL_SLABS + r // P] = sw[sel]
            toksp_host[r % P, hh * SPILL_SLABS + r // P] = (
                st[sel] - hh * HTOK
            ).astype(np.int16)

        in_maps.append({
            "idx16": idx16_host,
            "idxsp": idxsp_host,
            "wslot": wslot_host,
            "tokloc": tokloc_host,
            "wsp": wsp_host,
            "toksp": toksp_host,
            "iota64": iota64_h,
            "iota512": iota512_h,
            "wout": wout_host,
            "bias": bias_host,
            "ones": ones_host,
        })
    return in_maps


def kernel(weights, indexes, knowledge_base, w_out, b_out):
    from concourse.bass_utils import run_bass_kernel_spmd

    if "nc" not in _CACHE:
        _CACHE["nc"] = _build_bass()
    nc = _CACHE["nc"]

    kb_host = np.ascontiguousarray(knowledge_base, dtype=np.float32)
    in_maps = _host_prep(weights, indexes, w_out, b_out)
    for m in in_maps:
        m["kb"] = kb_host

    res = run_bass_kernel_spmd(nc, in_maps, list(range(NCORES)))
    out = np.concatenate([res.results[c]["out"] for c in range(NCORES)], axis=0)
    return out.reshape(B, T, E).astype(np.float32)
